# revision 10
# baseline (speedup 1.0000x reference)
"""MiniGPT forward (8 layers, D=1024, H=16, B=4, T=1024, V=32000) on 8 trn2 cores.

Sharding: 4 tensor-parallel pairs x data-parallel batch.
Core c: batch b=c//2, TP half g=c%2 (8 heads, 2048 FFN hidden, 16000 vocab).
Activations live transposed [feature, token] so natural-layout weights are lhsT.
Matmuls in float32r (fp22 mantissa, full PE rate at free-dim>=256).
Pairwise AllReduce after attention out-proj and FFN W2 partials.
"""
import os
import sys

sys.path.insert(0, "/opt/trn_rl_repo")

import numpy as np
from ml_dtypes import bfloat16 as ml_bf16

import concourse.bass as bass
import concourse.mybir as mybir
import concourse.tile as tile
from concourse import bacc
from concourse.bass_utils import run_bass_kernel_spmd

F32 = mybir.dt.float32
R32 = mybir.dt.float32r
AF = mybir.ActivationFunctionType
OP = mybir.AluOpType

V, D, H, L = 32000, 1024, 16, 8
HD = D // H          # 64
B, T = 4, 1024
EPS = 1e-5
HL = H // 2          # 8 local heads
FFL = 2 * D          # 2048 local ffn hidden
VL = V // 2          # 16000 local vocab
DK = D // 128        # 8 k-tiles over D
NMASK = -60000.0
GROUPS = [[0, 1], [2, 3], [4, 5], [6, 7]]

_PROG = None


def _build_program():
    nc = bacc.Bacc("TRN2", target_bir_lowering=False, debug=False, num_devices=8)

    def din(name, shape, dt=F32):
        return nc.dram_tensor(name, shape, dt, kind="ExternalInput").ap()

    x0t = din("x0t", [D, T])
    wq = din("wq", [L, 4, 128, DK, 128])
    wk = din("wk", [L, 4, 128, DK, 128])
    wv = din("wv", [L, 128, DK, 512])
    wo = din("wo", [L, 4, 128, DK, 128])
    w1 = din("w1", [L, 16, 128, DK, 128])
    w2 = din("w2", [L, 8, 2, 128, DK, 128])
    wh = din("wh", [128, DK, VL])
    masks = din("masks", [4, 128, 512], mybir.dt.bfloat16)
    ones = din("ones", [128, 128])
    ln1g = din("ln1g", [L, 128, DK])
    ln1b = din("ln1b", [L, 128, DK])
    ln2g = din("ln2g", [L, 128, DK])
    ln2b = din("ln2b", [L, 128, DK])
    lnfg = din("lnfg", [128, DK])
    lnfb = din("lnfb", [128, DK])
    b1c = din("b1c", [L, 128, 16])
    boc = din("boc", [L, 128, DK])
    b2c = din("b2c", [L, 128, DK])
    bhb = din("bhb", [128, VL])
    logits = nc.dram_tensor("logits", [T, VL], F32, kind="ExternalOutput").ap()

    with tile.TileContext(nc) as tc:
        with (
            tc.tile_pool(name="pers", bufs=1) as pers,
            tc.tile_pool(name="xn", bufs=1) as xnp,
            tc.tile_pool(name="big", bufs=16) as bigp,
            tc.tile_pool(name="wA", bufs=4) as wap,
            tc.tile_pool(name="whv", bufs=1) as whvp,
            tc.tile_pool(name="st", bufs=4) as stp,
            tc.tile_pool(name="rows", bufs=5) as rowp,
            tc.tile_pool(name="lnc", bufs=2) as lncp,
            tc.tile_pool(name="ar", bufs=1) as arp,
            tc.tile_pool(name="lo", bufs=2) as lop,
            tc.tile_pool(name="bh", bufs=1) as bhp,
            tc.tile_pool(name="psa", bufs=3, space="PSUM") as psa,
            tc.tile_pool(name="psb", bufs=3, space="PSUM") as psb,
            tc.tile_pool(name="psr", bufs=2, space="PSUM") as psr,
            tc.tile_pool(name="dram", bufs=4, space="DRAM") as dramp,
            nc.allow_low_precision(reason="fp32r activation pipeline"),
        ):
            # ---- persistent constants ----
            ones_sb = pers.tile([128, 128], R32, name="ones_sb")
            nc.sync.dma_start(ones_sb[:], ones[:].bitcast(R32))
            mask_sb = []
            for r in range(4):
                m = pers.tile([128, 512], mybir.dt.bfloat16, name=f"mask{r}", tag=f"mask{r}")
                nc.sync.dma_start(m[:], masks[r])
                mask_sb.append(m)
            lnfg_sb = pers.tile([128, DK], F32, name="lnfg_sb")
            lnfb_sb = pers.tile([128, DK], F32, name="lnfb_sb")
            nc.sync.dma_start(lnfg_sb[:], lnfg[:])
            nc.sync.dma_start(lnfb_sb[:], lnfb[:])

            # ---- residual stream x (transposed, f32r) ----
            x_sb = []
            for k in range(DK):
                xk = pers.tile([128, T], R32, name=f"x{k}", tag=f"x{k}")
                nc.sync.dma_start(xk[:], x0t[k * 128:(k + 1) * 128, :].bitcast(R32))
                x_sb.append(xk)

            # ---- v_ext: [token-tile][128, HL, 65]; col 64 = ones (denominator) ----
            v_ext = []
            for tt in range(8):
                vt = pers.tile([128, HL, 65], R32, name=f"vext{tt}", tag=f"vext{tt}")
                nc.sync.dma_start(vt[:, :, 64:65], ones[:, 0:HL, None].bitcast(R32))
                v_ext.append(vt)

            xn = [None] * DK  # layernorm output tiles (rotating slots)

            def layernorm_chunk(src, c, g_sb, b_sb):
                """LN over partition-dim D for token chunk c; writes xn[k][:, cs]."""
                cs = slice(c * 512, (c + 1) * 512)
                ps_s = psr.tile([1, 512], F32, name="ps_s", tag="psr")
                ps_q = psr.tile([1, 512], F32, name="ps_q", tag="psr")
                for k in range(DK):
                    nc.tensor.matmul(ps_s[:], ones_sb[:, 0:1], src[k][:, cs],
                                     start=(k == 0), stop=(k == DK - 1))
                for k in range(DK):
                    sq = stp.tile([128, 512], R32, name="sq", tag="st")
                    nc.vector.tensor_tensor(sq[:], src[k][:, cs], src[k][:, cs], OP.mult)
                    nc.tensor.matmul(ps_q[:], ones_sb[:, 0:1], sq[:],
                                     start=(k == 0), stop=(k == DK - 1))
                mu = rowp.tile([1, 512], F32, name="mu", tag="row")
                nc.vector.tensor_scalar_mul(mu[:], ps_s[:], 1.0 / D)
                ex2 = rowp.tile([1, 512], F32, name="ex2", tag="row")
                nc.vector.tensor_scalar(ex2[:], ps_q[:], 1.0 / D, EPS,
                                        op0=OP.mult, op1=OP.add)
                var = rowp.tile([1, 512], F32, name="var", tag="row")
                nc.vector.tensor_tensor(var[:], mu[:], mu[:], OP.mult)
                nc.vector.tensor_tensor(var[:], ex2[:], var[:], OP.subtract)
                std = rowp.tile([1, 512], F32, name="std", tag="row")
                nc.scalar.sqrt(std[:], var[:])
                rstd = rowp.tile([1, 512], R32, name="rstd", tag="row")
                nc.vector.reciprocal(rstd[:], std[:])
                mus = rowp.tile([1, 512], R32, name="mus", tag="row")
                nc.vector.tensor_tensor(mus[:], mu[:], rstd[:], OP.mult)
                bc_r = psb.tile([128, 512], F32, name="bc_r", tag="pb")
                nc.tensor.matmul(bc_r[:], ones_sb[0:1, :], rstd[:], start=True, stop=True)
                bc_m = psb.tile([128, 512], F32, name="bc_m", tag="pb")
                nc.tensor.matmul(bc_m[:], ones_sb[0:1, :], mus[:], start=True, stop=True)
                for k in range(DK):
                    t1 = stp.tile([128, 512], R32, name="t1", tag="st")
                    nc.vector.tensor_tensor(t1[:], src[k][:, cs], bc_r[:], OP.mult)
                    nc.vector.tensor_tensor(t1[:], t1[:], bc_m[:], OP.subtract)
                    nc.vector.tensor_scalar(xn[k][:, cs], t1[:],
                                            g_sb[:, k:k + 1], b_sb[:, k:k + 1],
                                            op0=OP.mult, op1=OP.add)

            def alloc_xn():
                for k in range(DK):
                    xn[k] = xnp.tile([128, T], R32, name=f"xn{k}", tag=f"xn{k}")

            def ln_params(gd, bd, l):
                g_sb = lncp.tile([128, DK], F32, name="g_sb", tag="lng")
                b_sb = lncp.tile([128, DK], F32, name="b_sb", tag="lnb")
                nc.sync.dma_start(g_sb[:], gd[l])
                nc.sync.dma_start(b_sb[:], bd[l])
                return g_sb, b_sb

            # ================= layers =================
            for l in range(L):
                # ---- LN1 ----
                g1, b1s = ln_params(ln1g, ln1b, l)
                alloc_xn()
                for c in range(2):
                    layernorm_chunk(x_sb, c, g1, b1s)

                # ---- QKV ----
                qT, kT = [], []
                for p in range(4):
                    wq_t = wap.tile([128, DK, 128], R32, name="wq_t", tag="wA")
                    nc.sync.dma_start(wq_t[:], wq[l, p].bitcast(R32))
                    q_p = bigp.tile([128, T], R32, name=f"qT{p}", tag="big")
                    for c in range(2):
                        cs = slice(c * 512, (c + 1) * 512)
                        ps = psa.tile([128, 512], F32, name="ps_q", tag="pa")
                        for k in range(DK):
                            nc.tensor.matmul(ps[:], wq_t[:, k, :], xn[k][:, cs],
                                             start=(k == 0), stop=(k == DK - 1))
                        nc.any.tensor_copy(q_p[:, cs], ps[:])
                    qT.append(q_p)
                for p in range(4):
                    wk_t = wap.tile([128, DK, 128], R32, name="wk_t", tag="wA")
                    nc.sync.dma_start(wk_t[:], wk[l, p].bitcast(R32))
                    k_p = bigp.tile([128, T], R32, name=f"kT{p}", tag="big")
                    for c in range(2):
                        cs = slice(c * 512, (c + 1) * 512)
                        ps = psa.tile([128, 512], F32, name="ps_k", tag="pa")
                        for k in range(DK):
                            nc.tensor.matmul(ps[:], wk_t[:, k, :], xn[k][:, cs],
                                             start=(k == 0), stop=(k == DK - 1))
                        nc.any.tensor_copy(k_p[:, cs], ps[:])
                    kT.append(k_p)
                wv_t = whvp.tile([128, DK, 512], R32, name="wv_t", tag="whv")
                nc.sync.dma_start(wv_t[:], wv[l].bitcast(R32))
                for tt in range(8):
                    ts_ = slice(tt * 128, (tt + 1) * 128)
                    ps = psa.tile([128, 512], F32, name="ps_v", tag="pa")
                    for k in range(DK):
                        nc.tensor.matmul(ps[:], xn[k][:, ts_], wv_t[:, k, :],
                                         start=(k == 0), stop=(k == DK - 1))
                    nc.any.tensor_copy(
                        v_ext[tt][:, :, 0:64],
                        ps[:].rearrange("p (h e) -> p h e", h=HL))

                # ---- attention ----
                oT = []
                for p in range(4):
                    o_p = bigp.tile([128, T], R32, name=f"oT{p}", tag="big")
                    oT.append(o_p)
                for hl in range(HL):
                    p, off = hl // 2, (hl % 2) * 64
                    for c in range(2):
                        cs = slice(c * 512, (c + 1) * 512)
                        nj = 4 * c + 4
                        ps_o = psb.tile([65, 512], F32, name="ps_o", tag="pb")
                        for j in range(nj):
                            ps_st = psa.tile([128, 512], F32, name="ps_st", tag="pa")
                            nc.tensor.matmul(
                                ps_st[:],
                                kT[p][off:off + 64, j * 128:(j + 1) * 128],
                                qT[p][off:off + 64, cs],
                                start=True, stop=True)
                            et = stp.tile([128, 512], R32, name="et", tag="st")
                            r = j - 4 * c
                            if r >= 0:
                                tm = stp.tile([128, 512], F32, name="tm", tag="st")
                                nc.vector.tensor_tensor(tm[:], ps_st[:],
                                                        mask_sb[r][:], OP.add)
                                nc.scalar.activation(et[:], tm[:], AF.Exp, scale=0.125)
                            else:
                                nc.scalar.activation(et[:], ps_st[:], AF.Exp,
                                                     scale=0.125)
                            nc.tensor.matmul(ps_o[:], v_ext[j][:, hl, :], et[:],
                                             start=(j == 0), stop=(j == nj - 1))
                        rec = rowp.tile([1, 512], R32, name="rec", tag="row")
                        nc.vector.reciprocal(rec[:], ps_o[64:65, :])
                        bc = psb.tile([64, 512], F32, name="bc_o", tag="pb")
                        nc.tensor.matmul(bc[:], ones_sb[0:1, 0:64], rec[:],
                                         start=True, stop=True)
                        ot = stp.tile([64, 512], R32, name="ot", tag="st")
                        nc.any.tensor_copy(ot[:], ps_o[0:64, :])
                        nc.vector.tensor_tensor(oT[p][off:off + 64, cs], ot[:],
                                                bc[:], OP.mult)

                # ---- out-proj + AllReduce + residual + LN2 (chunk-pipelined) ----
                bo_sb = lncp.tile([128, DK], F32, name="bo_sb", tag="bo")
                nc.sync.dma_start(bo_sb[:], boc[l])
                wo_t = []
                for k4 in range(4):
                    wt = wap.tile([128, DK, 128], R32, name="wo_t", tag="wA")
                    nc.sync.dma_start(wt[:], wo[l, k4].bitcast(R32))
                    wo_t.append(wt)
                g2, b2s = ln_params(ln2g, ln2b, l)
                alloc_xn()
                for c in range(2):
                    cs = slice(c * 512, (c + 1) * 512)
                    bnc_i = dramp.tile([DK, 128, 512], F32, name="bnc_ai", tag="bnci")
                    bnc_o = dramp.tile([DK, 128, 512], F32, name="bnc_ao", tag="bnco")
                    for m in range(DK):
                        ps = psa.tile([128, 512], F32, name="ps_ao", tag="pa")
                        for k4 in range(4):
                            nc.tensor.matmul(ps[:], wo_t[k4][:, m, :], oT[k4][:, cs],
                                             start=(k4 == 0), stop=(k4 == 3))
                        stg = stp.tile([128, 512], F32, name="stg_a", tag="st")
                        nc.any.tensor_copy(stg[:], ps[:])
                        nc.sync.dma_start(bnc_i[m], stg[:])
                    nc.gpsimd.collective_compute(
                        "AllReduce", OP.add, replica_groups=GROUPS,
                        ins=[bnc_i.opt()], outs=[bnc_o.opt()])
                    for k in range(DK):
                        ar = arp.tile([128, 512], F32, name="ar_a", tag="ar")
                        nc.sync.dma_start(ar[:], bnc_o[k])
                        nc.vector.tensor_scalar(ar[:], ar[:], bo_sb[:, k:k + 1], None,
                                                op0=OP.add)
                        nc.vector.tensor_tensor(x_sb[k][:, cs], x_sb[k][:, cs],
                                                ar[:], OP.add)
                    layernorm_chunk(x_sb, c, g2, b2s)

                # ---- FFN ----
                b1_sb = lncp.tile([128, 16], F32, name="b1_sb", tag="b1")
                nc.sync.dma_start(b1_sb[:], b1c[l])
                hblk = []
                for m16 in range(16):
                    w1_t = wap.tile([128, DK, 128], R32, name="w1_t", tag="wA")
                    nc.sync.dma_start(w1_t[:], w1[l, m16].bitcast(R32))
                    hm = bigp.tile([128, T], R32, name=f"h{m16}", tag="big")
                    for c in range(2):
                        cs = slice(c * 512, (c + 1) * 512)
                        ps = psa.tile([128, 512], F32, name="ps_h", tag="pa")
                        for k in range(DK):
                            nc.tensor.matmul(ps[:], w1_t[:, k, :], xn[k][:, cs],
                                             start=(k == 0), stop=(k == DK - 1))
                        nc.scalar.activation(hm[:, cs], ps[:], AF.Gelu,
                                             bias=b1_sb[:, m16:m16 + 1])
                    hblk.append(hm)
                b2_sb = lncp.tile([128, DK], F32, name="b2_sb", tag="bo")
                nc.sync.dma_start(b2_sb[:], b2c[l])
                bnc_fi, bnc_fo = [], []
                for c in range(2):
                    bnc_fi.append(dramp.tile([DK, 128, 512], F32,
                                             name="bnc_fi", tag="bnci"))
                    bnc_fo.append(dramp.tile([DK, 128, 512], F32,
                                             name="bnc_fo", tag="bnco"))
                for m in range(DK):
                    w2a = wap.tile([128, DK, 128], R32, name="w2a", tag="wA")
                    nc.sync.dma_start(w2a[:], w2[l, m, 0].bitcast(R32))
                    w2b = wap.tile([128, DK, 128], R32, name="w2b", tag="wA")
                    nc.sync.dma_start(w2b[:], w2[l, m, 1].bitcast(R32))
                    for c in range(2):
                        cs = slice(c * 512, (c + 1) * 512)
                        ps = psa.tile([128, 512], F32, name="ps_y", tag="pa")
                        for k8 in range(DK):
                            nc.tensor.matmul(ps[:], w2a[:, k8, :], hblk[k8][:, cs],
                                             start=(k8 == 0), stop=False)
                        for k8 in range(DK):
                            nc.tensor.matmul(ps[:], w2b[:, k8, :], hblk[8 + k8][:, cs],
                                             start=False, stop=(k8 == DK - 1))
                        stg = stp.tile([128, 512], F32, name="stg_f", tag="st")
                        nc.any.tensor_copy(stg[:], ps[:])
                        nc.sync.dma_start(bnc_fi[c][m], stg[:])
                for c in range(2):
                    cs = slice(c * 512, (c + 1) * 512)
                    nc.gpsimd.collective_compute(
                        "AllReduce", OP.add, replica_groups=GROUPS,
                        ins=[bnc_fi[c].opt()], outs=[bnc_fo[c].opt()])
                    for k in range(DK):
                        ar = arp.tile([128, 512], F32, name="ar_f", tag="ar")
                        nc.sync.dma_start(ar[:], bnc_fo[c][k])
                        nc.vector.tensor_scalar(ar[:], ar[:], b2_sb[:, k:k + 1], None,
                                                op0=OP.add)
                        nc.vector.tensor_tensor(x_sb[k][:, cs], x_sb[k][:, cs],
                                                ar[:], OP.add)

            # ================= final LN + head =================
            alloc_xn()
            for c in range(2):
                layernorm_chunk(x_sb, c, lnfg_sb, lnfb_sb)
            nch = (VL + 511) // 512
            for n in range(nch):
                n0 = n * 512
                nw = min(512, VL - n0)
                wh_t = whvp.tile([128, DK, 512], R32, name="wh_t", tag="whv")
                nc.sync.dma_start(wh_t[:, :, :nw], wh[:, :, n0:n0 + nw].bitcast(R32))
                bh_t = bhp.tile([128, 512], F32, name="bh_t", tag="bh")
                nc.sync.dma_start(bh_t[:, :nw], bhb[:, n0:n0 + nw])
                for m in range(8):
                    ms = slice(m * 128, (m + 1) * 128)
                    ps = psa.tile([128, 512], F32, name="ps_l", tag="pa")
                    for k in range(DK):
                        nc.tensor.matmul(ps[:, :nw], xn[k][:, ms], wh_t[:, k, :nw],
                                         start=(k == 0), stop=(k == DK - 1))
                    lo = lop.tile([128, 512], F32, name="lo_t", tag="lo")
                    nc.vector.tensor_tensor(lo[:, :nw], ps[:, :nw], bh_t[:, :nw],
                                            OP.add)
                    nc.sync.dma_start(logits[ms, n0:n0 + nw], lo[:, :nw])

    nc.compile()
    return nc


def _get_prog():
    global _PROG
    if _PROG is None:
        _PROG = _build_program()
    return _PROG


def _prep_core(c, idx, tok_emb, pos_emb, Wq, Wk, Wv, Wo, bo, ln1_g, ln1_b,
               ln2_g, ln2_b, W1, b1, W2, b2, lnf_g, lnf_b, Whead, bhead):
    b, g = c // 2, c % 2
    f32 = np.float32
    out = {}
    x0 = tok_emb[idx[b]] + pos_emb[:T]
    out["x0t"] = np.ascontiguousarray(x0.T, dtype=f32)

    hs = slice(g * HL, (g + 1) * HL)

    def qk_tiles(W):
        # W: [L,H,D,HD] -> [L,4,128,DK,128] (pair-grouped lhsT, partition-major)
        r = np.empty((L, 4, 128, DK, 128), dtype=f32)
        for l in range(L):
            for p in range(4):
                pair = W[l, g * HL + 2 * p: g * HL + 2 * p + 2]   # [2,D,HD]
                m = pair.transpose(1, 0, 2).reshape(D, 128)        # [D,128]
                r[l, p] = m.reshape(DK, 128, 128).transpose(1, 0, 2)
        return r

    out["wq"] = qk_tiles(Wq)
    out["wk"] = qk_tiles(Wk)
    r = np.empty((L, 128, DK, 512), dtype=f32)
    for l in range(L):
        m = Wv[l, hs].transpose(1, 0, 2).reshape(D, 512)
        r[l] = m.reshape(DK, 128, 512).transpose(1, 0, 2)
    out["wv"] = r
    r = np.empty((L, 4, 128, DK, 128), dtype=f32)
    for l in range(L):
        m = Wo[l, 512 * g:512 * (g + 1), :]                        # [512,D]
        # r[l, k4, p, mo, mc] = m[k4*128+p, mo*128+mc]
        r[l] = m.reshape(4, 128, DK, 128)
    out["wo"] = r
    r = np.empty((L, 16, 128, DK, 128), dtype=f32)
    for l in range(L):
        m = W1[l][:, g * FFL:(g + 1) * FFL]                        # [D,2048]
        r[l] = m.reshape(DK, 128, 16, 128).transpose(2, 1, 0, 3)
    out["w1"] = r
    r = np.empty((L, 8, 2, 128, DK, 128), dtype=f32)
    for l in range(L):
        m = W2[l][g * FFL:(g + 1) * FFL, :]                        # [2048,D]
        t = m.reshape(2, 8, 128, 8, 128)                           # [half,k8,p,mo,mc]
        # r[l, mo, half, p, k8, mc] = t[half, k8, p, mo, mc]
        r[l] = t.transpose(3, 0, 2, 1, 4)
    out["w2"] = r
    whs = Whead[:, g * VL:(g + 1) * VL]                            # [D,VL]
    out["wh"] = np.ascontiguousarray(
        whs.reshape(DK, 128, VL).transpose(1, 0, 2), dtype=f32)

    mk = np.zeros((4, 128, 512), dtype=f32)
    for rr in range(4):
        pp = np.arange(128)[:, None] + 128 * rr
        ff = np.arange(512)[None, :]
        mk[rr] = np.where(pp <= ff, 0.0, NMASK)
    out["masks"] = mk.astype(ml_bf16)
    out["ones"] = np.ones((128, 128), dtype=f32)

    def col(v, n):
        return np.ascontiguousarray(v.reshape(n, 128).T, dtype=f32)

    out["ln1g"] = np.stack([col(ln1_g[l], DK) for l in range(L)])
    out["ln1b"] = np.stack([col(ln1_b[l], DK) for l in range(L)])
    out["ln2g"] = np.stack([col(ln2_g[l], DK) for l in range(L)])
    out["ln2b"] = np.stack([col(ln2_b[l], DK) for l in range(L)])
    out["lnfg"] = col(lnf_g, DK)
    out["lnfb"] = col(lnf_b, DK)
    out["b1c"] = np.stack([col(b1[l][g * FFL:(g + 1) * FFL], 16) for l in range(L)])
    out["boc"] = np.stack([col(bo[l], DK) for l in range(L)])
    out["b2c"] = np.stack([col(b2[l], DK) for l in range(L)])
    out["bhb"] = np.ascontiguousarray(
        np.broadcast_to(bhead[g * VL:(g + 1) * VL][None, :], (128, VL)), dtype=f32)
    return out


def kernel(**inputs):
    inputs = {k: np.asarray(v) for k, v in inputs.items()}
    prog = _get_prog()
    in_maps = [_prep_core(c, **inputs) for c in range(8)]
    trace = bool(int(os.environ.get("KTRACE", "0")))
    res = run_bass_kernel_spmd(prog, in_maps, core_ids=list(range(8)), trace=trace)
    if trace:
        kernel.last_exec_time_ns = res.exec_time_ns
        kernel.last_results = res
    out = np.empty((B, T, V), dtype=np.float32)
    for c in range(8):
        b, g = c // 2, c % 2
        out[b, :, g * VL:(g + 1) * VL] = res.results[c]["logits"]
    return out
